# revision 1
# baseline (speedup 1.0000x reference)
"""Trainium2 Bass kernel for nn_ComplexNN (3-layer MLP, blended tanh act).

  h1 = blend_act(x @ W1 + b1);  blend_act(z) = z>0 ? 0.9z+0.1tanh(z) : 0.5tanh(z)
  h2 = relu(h1 @ W2 + b2)
  out = h2 @ W3 + b3

Data-parallel over 8 NeuronCores: each core takes 4096 rows of x, weights
replicated. Fully fused on-chip; matmuls in bf16 with fp32 PSUM accumulate.

Layout: activations are kept feature-on-partitions (h1^T, h2^T) so each
matmul's contraction dim lands on partitions with no intermediate
transposes. x is cast fp32->bf16 via SWDGE DMA (DRAM->DRAM, k-major
slices) then DMA-xbar-transposed (DRAM->SBUF). The final layer keeps the
transposed orientation: out^T [10, 4096] goes to DRAM and the host
transposes during the unshard/gather step.

blend_act decomposition (t = tanh(z)):
  blend(z) = 0.9*relu(z) + 0.1*t + 0.4*min(t, 0)
ACT: t = Tanh(psum + b1);  a = Relu(0.9*psum + 0.9*b1)
DVE: m = (t min 0)*0.4 ;  u = 0.1*t + a (STT);  h1 = u + m
"""

import sys

sys.path.insert(0, "/opt/trn_rl_repo")

import ml_dtypes
import numpy as np

import concourse.bass as bass
import concourse.mybir as mybir
import concourse.tile as tile
from concourse import bacc
from concourse.bass_utils import run_bass_kernel_spmd

N_CORES = 8
B, D, H, H2, C = 32768, 512, 1024, 512, 10
BL = B // N_CORES  # rows per core = 4096
# Batch chunk sizes: small first chunks fill the cast->xbar->matmul pipeline
# quickly; later chunks are wide to amortize fixed per-instruction costs; a
# small final chunk shortens the mm2->mm3->store drain tail.
CHUNKS = [256, 256, 512, 1024, 1024, 768, 256]
assert sum(CHUNKS) == BL
KD = D // 128      # 4  k-tiles for mm1
KH = H // 128      # 8  k-tiles for mm2 / h-tiles of h1
KH2 = H2 // 128    # 4  k-tiles for mm3 / h2-tiles of h2

F32 = mybir.dt.float32
BF16 = mybir.dt.bfloat16
AF = mybir.ActivationFunctionType
ALU = mybir.AluOpType


def _body(ctx, tc, outs, ins):
    nc = tc.nc
    x, w1, w2, w3, b1c, b1s, b2c, b3c = ins
    (outT,) = outs

    wpool = ctx.enter_context(tc.tile_pool(name="weights", bufs=1))
    xpool = ctx.enter_context(tc.tile_pool(name="xT", bufs=2 * KD))
    h1pool = ctx.enter_context(tc.tile_pool(name="h1T", bufs=2 * KH))
    h2pool = ctx.enter_context(tc.tile_pool(name="h2T", bufs=2 * KH2))
    tpool = ctx.enter_context(tc.tile_pool(name="tmp", bufs=3))
    opool = ctx.enter_context(tc.tile_pool(name="ostage", bufs=2))
    mmpool = ctx.enter_context(tc.tile_pool(name="mm", bufs=3, space="PSUM"))
    mm3pool = ctx.enter_context(tc.tile_pool(name="mm3", bufs=1, space="PSUM"))
    xbd = ctx.enter_context(tc.tile_pool(name="xbd", bufs=2, space="DRAM"))

    # resident weights / biases (scalar-engine HWDGE queue, so the sync
    # queue is free for the xbar transposes)
    w1s = wpool.tile([128, KD * H], BF16)     # w1s[p, k*H + h]  = W1[k*128+p, h]
    w2s = wpool.tile([128, KH * H2], BF16)    # w2s[p, k*H2 + m] = W2[k*128+p, m]
    w3s = wpool.tile([128, KH2 * C], BF16)    # w3s[p, k*C + c]  = W3[k*128+p, c]
    b1cs = wpool.tile([128, KH], F32)         # b1cs[p, i] = b1[i*128+p]
    b1ss = wpool.tile([128, KH], F32)         # 0.9 * b1
    b2cs = wpool.tile([128, KH2], F32)
    b3cs = wpool.tile([C, 1], F32)            # b3 as per-partition column
    # Weight loads are interleaved into the SWDGE queue AFTER the first
    # chunk's cast (see below): the single SWDGE queue is the ingest
    # critical path, and the first xbar transpose can only start once the
    # first cast transfer completes.
    def load_weights():
        nc.gpsimd.dma_start(out=w1s[:], in_=w1[:])
        nc.gpsimd.dma_start(out=b1cs[:], in_=b1c[:])
        nc.gpsimd.dma_start(out=b1ss[:], in_=b1s[:])
        nc.gpsimd.dma_start(out=w2s[:], in_=w2[:])
        nc.gpsimd.dma_start(out=b2cs[:], in_=b2c[:])
        nc.gpsimd.dma_start(out=w3s[:], in_=w3[:])
        nc.gpsimd.dma_start(out=b3cs[:], in_=b3c[:])

    row0 = 0
    for c, NB in enumerate(CHUNKS):
        bs_cols = [slice(s, min(s + 512, NB)) for s in range(0, NB, 512)]
        rows = slice(row0, row0 + NB)
        row0 += NB

        # contiguous fp32->bf16 cast of the whole chunk (SWDGE DRAM->DRAM),
        # then per-k-slice xbar transposes into SBUF, spread over both
        # HWDGE queues (sync + scalar).
        xb = xbd.tile([max(CHUNKS), D], BF16, tag="xb", name="xb")[:NB]
        nc.gpsimd.dma_start(out=xb[:], in_=x[rows, :])
        if c == 0:
            load_weights()
        xT = []
        for k in range(KD):
            xt = xpool.tile([128, max(CHUNKS)], BF16, tag="xt", name="xt")[:, :NB]
            nc.sync.dma_start(out=xt[:], in_=xb[:, k * 128 : (k + 1) * 128], transpose=True)
            xT.append(xt)

        # ---- mm1 + blend_act ----  h1T[i] [128 h, NB b]
        h1T = []
        for i in range(KH):
            ps = mmpool.tile([128, NB], F32, tag="ps")
            for cols in bs_cols:
                for k in range(KD):
                    nc.tensor.matmul(
                        ps[:, cols],
                        w1s[:, k * H + i * 128 : k * H + (i + 1) * 128],
                        xT[k][:, cols],
                        start=(k == 0),
                        stop=(k == KD - 1),
                    )
            t = tpool.tile([128, NB], BF16, tag="t")
            a = tpool.tile([128, NB], BF16, tag="a")
            nc.scalar.activation(t[:], ps[:], AF.Tanh, bias=b1cs[:, i : i + 1], scale=1.0)
            nc.scalar.activation(a[:], ps[:], AF.Relu, bias=b1ss[:, i : i + 1], scale=0.9)
            m = tpool.tile([128, NB], BF16, tag="m")
            u = tpool.tile([128, NB], BF16, tag="u")
            nc.vector.tensor_scalar(m[:], t[:], 0.0, 0.4, ALU.min, ALU.mult)
            nc.vector.scalar_tensor_tensor(u[:], t[:], 0.1, a[:], ALU.mult, ALU.add)
            h1 = h1pool.tile([128, NB], BF16, tag="h1")
            nc.vector.tensor_add(h1[:], u[:], m[:])
            h1T.append(h1)

        # ---- mm2 + relu (relu on DVE: ACT is the contended engine) ----
        h2T = []
        for j in range(KH2):
            ps2 = mmpool.tile([128, NB], F32, tag="ps")
            for cols in bs_cols:
                for k in range(KH):
                    nc.tensor.matmul(
                        ps2[:, cols],
                        w2s[:, k * H2 + j * 128 : k * H2 + (j + 1) * 128],
                        h1T[k][:, cols],
                        start=(k == 0),
                        stop=(k == KH - 1),
                    )
            h2 = h2pool.tile([128, NB], BF16, tag="h2")
            nc.scalar.activation(h2[:], ps2[:], AF.Relu, bias=b2cs[:, j : j + 1], scale=1.0)
            h2T.append(h2)

        # ---- mm3: out^T [10, NB] = W3^T @ h2 + b3 ----
        ps3 = mm3pool.tile([C, NB], F32, tag="ps3")
        for cols in bs_cols:
            for k in range(KH2):
                nc.tensor.matmul(
                    ps3[:, cols],
                    w3s[:, k * C : (k + 1) * C],
                    h2T[k][:, cols],
                    start=(k == 0),
                    stop=(k == KH2 - 1),
                )
        stage = opool.tile([C, NB], F32, tag="stage")
        nc.vector.tensor_scalar_add(stage[:], ps3[:], b3cs[:])
        # SWDGE (gpsimd) for the store: keeps the HWDGE rings transpose-only,
        # avoiding the DMATranspose<->DMACopy xbar-mode serialization.
        nc.gpsimd.dma_start(out=outT[:, rows], in_=stage[:])


_CACHED = None


def _build():
    global _CACHED
    if _CACHED is not None:
        return _CACHED
    nc = bacc.Bacc(
        "TRN2",
        target_bir_lowering=False,
        debug=False,
        enable_asserts=False,
        num_devices=N_CORES,
    )
    x = nc.dram_tensor("x", [BL, D], F32, kind="ExternalInput").ap()
    w1 = nc.dram_tensor("w1", [128, KD * H], BF16, kind="ExternalInput").ap()
    w2 = nc.dram_tensor("w2", [128, KH * H2], BF16, kind="ExternalInput").ap()
    w3 = nc.dram_tensor("w3", [128, KH2 * C], BF16, kind="ExternalInput").ap()
    b1c = nc.dram_tensor("b1c", [128, KH], F32, kind="ExternalInput").ap()
    b1s = nc.dram_tensor("b1s", [128, KH], F32, kind="ExternalInput").ap()
    b2c = nc.dram_tensor("b2c", [128, KH2], F32, kind="ExternalInput").ap()
    b3c = nc.dram_tensor("b3c", [C, 1], F32, kind="ExternalInput").ap()
    outT = nc.dram_tensor("outT", [C, BL], F32, kind="ExternalOutput").ap()

    from contextlib import ExitStack

    with tile.TileContext(nc) as tc, ExitStack() as ctx:
        _body(ctx, tc, [outT], [x, w1, w2, w3, b1c, b1s, b2c, b3c])
    nc.compile()
    _CACHED = nc
    return nc


def _prep_weights(W1, b1, W2, b2, W3, b3):
    bf = ml_dtypes.bfloat16
    w1h = np.ascontiguousarray(
        W1.astype(bf).reshape(KD, 128, H).transpose(1, 0, 2).reshape(128, KD * H)
    )
    w2h = np.ascontiguousarray(
        W2.astype(bf).reshape(KH, 128, H2).transpose(1, 0, 2).reshape(128, KH * H2)
    )
    w3h = np.ascontiguousarray(
        W3.astype(bf).reshape(KH2, 128, C).transpose(1, 0, 2).reshape(128, KH2 * C)
    )
    b1f = b1.astype(np.float32)
    b1ch = np.ascontiguousarray(b1f.reshape(KH, 128).T)
    b1sh = np.ascontiguousarray((0.9 * b1f).reshape(KH, 128).T)
    b2ch = np.ascontiguousarray(b2.astype(np.float32).reshape(KH2, 128).T)
    b3ch = np.ascontiguousarray(b3.astype(np.float32).reshape(C, 1))
    return w1h, w2h, w3h, b1ch, b1sh, b2ch, b3ch


def _make_in_maps(x, W1, b1, W2, b2, W3, b3):
    x = np.asarray(x, dtype=np.float32)
    w1h, w2h, w3h, b1ch, b1sh, b2ch, b3ch = _prep_weights(
        np.asarray(W1), np.asarray(b1), np.asarray(W2), np.asarray(b2),
        np.asarray(W3), np.asarray(b3),
    )
    return [
        {
            "x": np.ascontiguousarray(x[i * BL : (i + 1) * BL]),
            "w1": w1h, "w2": w2h, "w3": w3h,
            "b1c": b1ch, "b1s": b1sh, "b2c": b2ch, "b3c": b3ch,
        }
        for i in range(N_CORES)
    ]


def _gather(core_outs):
    return np.concatenate(
        [np.ascontiguousarray(o["outT"].T) for o in core_outs], axis=0
    ).astype(np.float32, copy=False)


def kernel(x, W1, b1, W2, b2, W3, b3):
    nc = _build()
    in_maps = _make_in_maps(x, W1, b1, W2, b2, W3, b3)
    res = run_bass_kernel_spmd(nc, in_maps, core_ids=list(range(N_CORES))).results
    return _gather(res)



# revision 14
# speedup vs baseline: 14158.4183x; 14158.4183x over previous
"""Trainium2 Bass kernel for nn_ComplexNN (3-layer MLP, blended tanh act).

  h1 = blend_act(x @ W1 + b1);  blend_act(z) = z>0 ? 0.9z+0.1tanh(z) : 0.5tanh(z)
  h2 = relu(h1 @ W2 + b2)
  out = h2 @ W3 + b3

Data-parallel over 8 NeuronCores: each core takes 4096 rows of x, weights
replicated. Fully fused on-chip; matmuls in bf16 with fp32 PSUM accumulate.

Ingest (per core): x is cast fp32->bf16 per column-group by SWDGE DMA
(DRAM->DRAM, contiguous both sides), then DMA-xbar-transposed DRAM->SBUF
per 128-feature slice ([NB,128] -> [128,NB]; DMA_TRANSPOSE costs ~1.2us of
sync-engine time FLAT regardless of size, so transposes must be few and
large -- 4 per group). The sync HWDGE ring carries ONLY transposes (no
xbar mode switches); weights/casts/stores ride the SWDGE queue, with the
per-(h-tile) weight slices interleaved between early casts in need order.

Compute is software-pipelined on the PE stream: mm1(g) | mm2(g-1) | mm3(g-2)
so each matmul stage's inputs are a full group old (ACT/DVE latency hidden).

blend_act decomposition (t = tanh(z)):
  blend(z) = 0.9*relu(z) + 0.1*t + 0.4*min(t, 0)
ACT:    t = Tanh(psum + b1);  a = Relu(0.9*psum + 0.9*b1)
DVE:    m = (t min 0)*0.4;  u = 0.1*t + a (STT);  h1 = u + m
mm2 relu+bias: j<2 on ACT (Relu), j>=2 on DVE ((psum+b2) max 0) -- splits
the relu load so neither ACT nor DVE exceeds the PE's ~117us of work.
Engine budget per core (~): PE 117us, ACT ~100us, DVE ~108us.
"""

import sys

sys.path.insert(0, "/opt/trn_rl_repo")

import ml_dtypes
import numpy as np

import concourse.bass as bass
import concourse.mybir as mybir
import concourse.tile as tile
from concourse import bacc
from concourse.bass_utils import run_bass_kernel_spmd

N_CORES = 8
B, D, H, H2, C = 32768, 512, 1024, 512, 10
BL = B // N_CORES  # rows per core = 4096
KD = D // 128      # 4  k-tiles for mm1
KH = H // 128      # 8  k-tiles for mm2 / h-tiles of h1
KH2 = H2 // 128    # 4  k-tiles for mm3 / h2-tiles of h2

# Cast chunks (one SWDGE DMA + 4 xbar transposes each; all staging and xT
# tiles are live simultaneously -- no pool recycling in the ingest path).
CASTS = [512, 1024, 1024, 1536]
assert sum(CASTS) == BL
COFF = [sum(CASTS[:c]) for c in range(len(CASTS))]
# Column groups (batch rows per matmul pass); each lies within one cast
# chunk. Small first groups start the PE early; small last groups shorten
# the drain tail.
GROUPS = [256, 256, 512, 512, 512, 512, 512, 512, 512]
assert sum(GROUPS) == BL
G = len(GROUPS)
GOFF = [sum(GROUPS[:g]) for g in range(G)]
# group -> (cast index, offset within cast)
GCAST = []
for g in range(G):
    c = next(i for i in range(len(CASTS)) if COFF[i] <= GOFF[g] < COFF[i] + CASTS[i])
    assert GOFF[g] + GROUPS[g] <= COFF[c] + CASTS[c]
    GCAST.append((c, GOFF[g] - COFF[c]))

F32 = mybir.dt.float32
BF16 = mybir.dt.bfloat16
AF = mybir.ActivationFunctionType
ALU = mybir.AluOpType


def _body(ctx, tc, outs, ins):
    nc = tc.nc
    x, w1, w2, w3, bpk, b3c = ins
    (outT,) = outs

    wpool = ctx.enter_context(tc.tile_pool(name="weights", bufs=1))
    xtpool = ctx.enter_context(tc.tile_pool(name="xT", bufs=1))
    h1pool = ctx.enter_context(tc.tile_pool(name="h1T", bufs=18))
    h2pool = ctx.enter_context(tc.tile_pool(name="h2T", bufs=10))
    tpool = ctx.enter_context(tc.tile_pool(name="tmp", bufs=4))
    opool = ctx.enter_context(tc.tile_pool(name="ostage", bufs=10))
    ps1pool = ctx.enter_context(tc.tile_pool(name="ps1", bufs=4, space="PSUM"))
    ps2pool = ctx.enter_context(tc.tile_pool(name="ps2", bufs=2, space="PSUM"))
    ps3pool = ctx.enter_context(tc.tile_pool(name="ps3", bufs=2, space="PSUM"))
    xbd = ctx.enter_context(tc.tile_pool(name="xbd", bufs=1, space="DRAM"))

    # resident weights / biases
    w1s = wpool.tile([128, KH * KD * 128], BF16)  # [p, i*512 + k*128 + c] = W1[k*128+p, i*128+c]
    w2s = wpool.tile([128, KH2 * KH * 128], BF16) # [p, j*1024 + k*128 + c] = W2[k*128+p, j*128+c]
    w3s = wpool.tile([128, KH2 * C], BF16)        # [p, k*C + c] = W3[k*128+p, c]
    bpks = wpool.tile([128, 2 * KH + KH2], F32)   # packed [b1 | 0.9*b1 | b2]
    b1cs = bpks[:, 0:KH]
    b1ss = bpks[:, KH : 2 * KH]
    b2cs = bpks[:, 2 * KH : 2 * KH + KH2]
    b3cs = wpool.tile([C, 1], F32)

    # ---- ingest: per-cast fp32->bf16 cast (SWDGE, DRAM->DRAM) + 4 big
    # xbar transposes each (sync ring, DRAM->SBUF, transpose-only ring).
    # Weights/biases interleave on the SWDGE queue between casts. No tile
    # is ever recycled in this path, so every transpose depends only on
    # its own cast's completion.
    xt = {}
    xb_next = {}

    def cast_chunk(c):
        R = CASTS[c]
        rows = slice(COFF[c], COFF[c] + R)
        xb = xb_next.get(c)
        if xb is None:
            xb = xbd.tile([R, D], BF16, tag=f"xb{c}", name="xb")
        nc.gpsimd.dma_start(out=xb[:], in_=x[rows, :])
        for k in range(KD):
            xt[(c, k)] = xtpool.tile([128, R], BF16, tag=f"xt{c}_{k}", name="xt")
            nc.sync.dma_start(
                out=xt[(c, k)][:],
                in_=xb[:, k * 128 : (k + 1) * 128],
                transpose=True,
            )

    # Phasing: the DMA engines serialize on xbar-mode transitions
    # (DMATranspose vs DMACopy, known HW bug), so copies and transposes
    # must never overlap. After each cast's transposes, a tiny SWDGE copy
    # reads the last transpose's output and writes into the NEXT cast's
    # staging tile -- the WAW dependency parks the whole SWDGE stream
    # (casts, weights, stores are all behind it in program order) until
    # the transpose window is over.
    def phase_gate(c):
        nxt = xbd.tile([CASTS[c + 1], D], BF16, tag=f"xb{c + 1}", name="xb")
        xb_next[c + 1] = nxt
        nc.gpsimd.dma_start(out=nxt[:1, :8], in_=xt[(c, KD - 1)][:1, :8])

    # w2/w3 ride the scalar HWDGE ring, dispatched before any transpose
    # exists -- plain copies on two queues never mode-conflict, and they
    # finish (~10us) before the first transpose window opens (~15us).
    nc.scalar.dma_start(out=w2s[:, 0:2048], in_=w2[:, 0:2048])
    nc.scalar.dma_start(out=w2s[:, 2048:4096], in_=w2[:, 2048:4096])
    nc.scalar.dma_start(out=w3s[:], in_=w3[:])
    nc.gpsimd.dma_start(out=bpks[:], in_=bpk[:])
    nc.gpsimd.dma_start(out=b3cs[:], in_=b3c[:])
    nc.gpsimd.dma_start(out=w1s[:], in_=w1[:])
    cast_chunk(0)
    phase_gate(0)
    cast_chunk(1)
    phase_gate(1)
    cast_chunk(2)
    phase_gate(2)
    cast_chunk(3)
    # final gate: stores (plain copies) must not overlap the last transpose
    # window. The gate writes one row across ALL groups' outT ranges so
    # every store picks up a WAW dependency on the last transpose.
    nc.gpsimd.dma_start(out=outT[0:1, 0:BL], in_=xt[(len(CASTS) - 1, KD - 1)][0:4, 0 : BL // 4])

    # ---- compute, software-pipelined PE stream ----
    h1T = {}
    h2T = {}

    def mm1(g):
        NB = GROUPS[g]
        c, coff = GCAST[g]
        for i in range(KH):
            ps = ps1pool.tile([128, NB], F32, tag="ps1")
            for k in range(KD):
                nc.tensor.matmul(
                    ps[:],
                    w1s[:, i * 512 + k * 128 : i * 512 + (k + 1) * 128],
                    xt[(c, k)][:, coff : coff + NB],
                    start=(k == 0),
                    stop=(k == KD - 1),
                )
            t = tpool.tile([128, NB], BF16, tag="t")
            a = tpool.tile([128, NB], BF16, tag="a")
            nc.scalar.activation(t[:], ps[:], AF.Tanh, bias=b1cs[:, i : i + 1], scale=1.0)
            nc.scalar.activation(a[:], ps[:], AF.Relu, bias=b1ss[:, i : i + 1], scale=0.9)
            m = tpool.tile([128, NB], BF16, tag="m")
            u = tpool.tile([128, NB], BF16, tag="u")
            nc.vector.tensor_scalar(m[:], t[:], 0.0, 0.4, ALU.min, ALU.mult)
            nc.vector.scalar_tensor_tensor(u[:], t[:], 0.1, a[:], ALU.mult, ALU.add)
            h1 = h1pool.tile([128, NB], BF16, tag="h1")
            nc.vector.tensor_add(h1[:], u[:], m[:])
            h1T[(g, i)] = h1

    def mm2(g):
        NB = GROUPS[g]
        for j in range(KH2):
            ps2 = ps2pool.tile([128, NB], F32, tag="ps2")
            for k in range(KH):
                nc.tensor.matmul(
                    ps2[:],
                    w2s[:, j * 1024 + k * 128 : j * 1024 + (k + 1) * 128],
                    h1T[(g, k)][:],
                    start=(k == 0),
                    stop=(k == KH - 1),
                )
            h2 = h2pool.tile([128, NB], BF16, tag="h2")
            if j < 2:
                nc.scalar.activation(h2[:], ps2[:], AF.Relu, bias=b2cs[:, j : j + 1], scale=1.0)
            else:
                nc.vector.tensor_scalar(h2[:], ps2[:], b2cs[:, j : j + 1], 0.0, ALU.add, ALU.max)
            h2T[(g, j)] = h2

    def mm3(g):
        NB = GROUPS[g]
        ps3 = ps3pool.tile([C, NB], F32, tag="ps3")
        for k in range(KH2):
            nc.tensor.matmul(
                ps3[:],
                w3s[:, k * C : (k + 1) * C],
                h2T[(g, k)][:],
                start=(k == 0),
                stop=(k == KH2 - 1),
            )
        stage = opool.tile([C, NB], F32, tag="stage")
        nc.vector.tensor_scalar_add(stage[:], ps3[:], b3cs[:])
        nc.gpsimd.dma_start(out=outT[:, GOFF[g] : GOFF[g] + NB], in_=stage[:])

    for g in range(G):
        mm1(g)
        if g >= 1:
            mm2(g - 1)
        if g >= 2:
            mm3(g - 2)
    mm2(G - 1)
    mm3(G - 2)
    mm3(G - 1)


_CACHED = None


def _build():
    global _CACHED
    if _CACHED is not None:
        return _CACHED
    nc = bacc.Bacc(
        "TRN2",
        target_bir_lowering=False,
        debug=False,
        enable_asserts=False,
        num_devices=N_CORES,
    )
    x = nc.dram_tensor("x", [BL, D], F32, kind="ExternalInput").ap()
    w1 = nc.dram_tensor("w1", [128, KH * KD * 128], BF16, kind="ExternalInput").ap()
    w2 = nc.dram_tensor("w2", [128, KH2 * KH * 128], BF16, kind="ExternalInput").ap()
    w3 = nc.dram_tensor("w3", [128, KH2 * C], BF16, kind="ExternalInput").ap()
    bpk = nc.dram_tensor("bpk", [128, 2 * KH + KH2], F32, kind="ExternalInput").ap()
    b3c = nc.dram_tensor("b3c", [C, 1], F32, kind="ExternalInput").ap()
    outT = nc.dram_tensor("outT", [C, BL], F32, kind="ExternalOutput").ap()

    from contextlib import ExitStack

    with tile.TileContext(nc) as tc, ExitStack() as ctx:
        _body(ctx, tc, [outT], [x, w1, w2, w3, bpk, b3c])
    nc.compile()
    _CACHED = nc
    return nc


def _prep_weights(W1, b1, W2, b2, W3, b3):
    bf = ml_dtypes.bfloat16
    # w1s[p, i*512 + k*128 + c] = W1[k*128+p, i*128+c]
    w1h = np.ascontiguousarray(
        W1.astype(bf).reshape(KD, 128, KH, 128).transpose(1, 2, 0, 3).reshape(128, KH * KD * 128)
    )
    # w2s[p, j*1024 + k*128 + c] = W2[k*128+p, j*128+c]
    w2h = np.ascontiguousarray(
        W2.astype(bf).reshape(KH, 128, KH2, 128).transpose(1, 2, 0, 3).reshape(128, KH2 * KH * 128)
    )
    # w3s[p, k*C + c] = W3[k*128+p, c]
    w3h = np.ascontiguousarray(
        W3.astype(bf).reshape(KH2, 128, C).transpose(1, 0, 2).reshape(128, KH2 * C)
    )
    b1f = b1.astype(np.float32)
    b1ch = np.ascontiguousarray(b1f.reshape(KH, 128).T)
    b1sh = np.ascontiguousarray((0.9 * b1f).reshape(KH, 128).T)
    b2ch = np.ascontiguousarray(b2.astype(np.float32).reshape(KH2, 128).T)
    b3ch = np.ascontiguousarray(b3.astype(np.float32).reshape(C, 1))
    bpkh = np.ascontiguousarray(np.concatenate([b1ch, b1sh, b2ch], axis=1))
    return w1h, w2h, w3h, bpkh, b3ch


def _make_in_maps(x, W1, b1, W2, b2, W3, b3):
    x = np.asarray(x, dtype=np.float32)
    w1h, w2h, w3h, bpkh, b3ch = _prep_weights(
        np.asarray(W1), np.asarray(b1), np.asarray(W2), np.asarray(b2),
        np.asarray(W3), np.asarray(b3),
    )
    return [
        {
            "x": np.ascontiguousarray(x[i * BL : (i + 1) * BL]),
            "w1": w1h, "w2": w2h, "w3": w3h,
            "bpk": bpkh, "b3c": b3ch,
        }
        for i in range(N_CORES)
    ]


def _gather(core_outs):
    return np.concatenate(
        [np.ascontiguousarray(o["outT"].T) for o in core_outs], axis=0
    ).astype(np.float32, copy=False)


def kernel(x, W1, b1, W2, b2, W3, b3):
    nc = _build()
    in_maps = _make_in_maps(x, W1, b1, W2, b2, W3, b3)
    res = run_bass_kernel_spmd(nc, in_maps, core_ids=list(range(N_CORES))).results
    return _gather(res)
